# revision 1
# baseline (speedup 1.0000x reference)
"""YOLO-style loss (nn_Loss_90142773608781) on 8 Trainium2 NeuronCores.

Strategy (data-parallel, per sharding hint):
- `output` [16384,7,7,30] is viewed as a flat cell table [802816, 30].
- Dense term (0.5 * sum(conf^2) over cols {4,9} of every cell): each core
  streams its 1/8 batch slice (12MB) through SBUF; ScalarE squares+reduces.
- Targeted terms: the 65536 targets are sorted by cell index and split
  evenly, 8192 per core (64 indirect-DMA gather chunks of 128 rows).
  Per-target loss terms are evaluated on VectorE/ScalarE with 4-wide
  packed ops; target-only quantities (box edges, signed sqrts, areas) are
  precomputed on the host into a small field tensor.
- Each core writes [128,2] partials (target-loss, conf-sq); host reduces.
"""

import sys

if "/opt/trn_rl_repo" not in sys.path:
    sys.path.append("/opt/trn_rl_repo")

import numpy as np

P = 128
D = 30
BATCH = 16384
GRID = 7
NTGT = 65536
CELLS = BATCH * GRID * GRID          # 802816
CELLS_CORE = CELLS // 8              # 100352
TGT_CORE = NTGT // 8                 # 8192
NCH = TGT_CORE // P                  # 64 gather chunks per core
PARTS = (16, 16, 16, 12, 4)          # target-math parts (chunks)
NF = 10                              # target fields (4 pairs + 2 singles)
STILES = 8                           # stream tiles per core
SW = CELLS_CORE // P // STILES       # cells per partition per stream tile (98)

_cache = {}


def _build():
    import concourse.bacc as bacc
    import concourse.tile as tile
    import concourse.mybir as mybir
    from concourse import bass

    F32 = mybir.dt.float32
    AL = mybir.AluOpType
    ACT = mybir.ActivationFunctionType
    X = mybir.AxisListType.X

    nc = bacc.Bacc("TRN2", target_bir_lowering=False, debug=False,
                   enable_asserts=False, num_devices=8)
    table = nc.dram_tensor("table", [CELLS, D], F32, kind="ExternalInput").ap()
    sl = nc.dram_tensor("sl", [CELLS_CORE, D], F32, kind="ExternalInput").ap()
    idx = nc.dram_tensor("idx", [P, NCH], mybir.dt.int32, kind="ExternalInput").ap()
    fld = nc.dram_tensor("fld", [P, NF * NCH + 20], F32, kind="ExternalInput").ap()
    out = nc.dram_tensor("partial", [P, 2], F32, kind="ExternalOutput").ap()

    vec, act = nc.vector, nc.scalar

    with tile.TileContext(nc) as tc:
        with (
            tc.tile_pool(name="cst", bufs=1) as cst,
            tc.tile_pool(name="grid", bufs=1) as gpool,
            tc.tile_pool(name="stream", bufs=3) as spool,
            tc.tile_pool(name="scr", bufs=2) as scr,
        ):
            # ---- setup loads (idx first: gathers depend on it) ----
            idx_t = cst.tile([P, NCH], mybir.dt.int32)
            nc.sync.dma_start(out=idx_t[:], in_=idx[:])
            fld_t = cst.tile([P, NF * NCH + 20], F32)
            nc.sync.dma_start(out=fld_t[:], in_=fld[:])

            eps_t = cst.tile([P, 1], F32)
            vec.memset(eps_t[:], 1e-6)
            neg1_t = cst.tile([P, 1], F32)
            vec.memset(neg1_t[:], -1.0)
            iotf = fld_t[:, NF * NCH:NF * NCH + 20]

            acc2 = cst.tile([P, 2], F32)         # col0: target loss, col1: conf sq
            vec.memset(acc2[:], 0.0)
            confp = cst.tile([P, STILES], F32)   # per-stream-tile conf partials

            # ---- gather: 64 indirect DMAs, 128 rows each ----
            grid_t = gpool.tile([P, NCH * D], F32)
            for k in range(NCH):
                nc.gpsimd.indirect_dma_start(
                    out=grid_t[:, k * D:(k + 1) * D],
                    out_offset=None,
                    in_=table[:],
                    in_offset=bass.IndirectOffsetOnAxis(ap=idx_t[:, k:k + 1], axis=0),
                )

            # ---- dense conf stream (squares+reduce on ScalarE) ----
            flat = sl.rearrange("(p x) c -> p (x c)", p=P)
            w = SW * D
            for i in range(STILES):
                st = spool.tile([P, w], F32, tag="st")
                nc.sync.dma_start(out=st[:], in_=flat[:, i * w:(i + 1) * w])
                st3 = st[:].rearrange("p (r c) -> p r c", c=D)
                conf = st3[:, :, 4:10:5]
                sq = spool.tile([P, SW * 2], F32, tag="sq")
                act.activation(sq[:].rearrange("p (r c) -> p r c", c=2), conf,
                               ACT.Square, accum_out=confp[:, i:i + 1])
            vec.tensor_reduce(out=acc2[:, 1:2], in_=confp[:], axis=X, op=AL.add)

            # ---- per-target math over chunk parts ----
            # field pair/single views: XY, LT, RB, SSQ pairs then AREA, CLS.
            def pair(i, k0, W):
                v = fld_t[:, :NF * NCH].rearrange("p (f k c) -> p f k c", f=5, c=2)
                return v[:, i, k0:k0 + W, :]

            def single(i, k0, W):
                v = fld_t[:, :NF * NCH].rearrange("p (f k) -> p f k", f=NF)
                return v[:, 8 + i, k0:k0 + W]

            k0 = 0
            for W in PARTS:
                g4 = grid_t[:].rearrange("p (k c) -> p k c", c=D)[:, k0:k0 + W, :]
                g5 = grid_t[:].rearrange("p (k b r) -> p k b r", b=6, r=5)[:, k0:k0 + W, :, :]
                xy = g5[:, :, 0:2, 0:2]      # [P,W,2box,2xy]
                wh = g5[:, :, 0:2, 2:4]
                cb = g5[:, :, 0:2, 4]        # [P,W,2]
                clsg = g4[:, :, 10:30]       # [P,W,20]

                def t4(tag):
                    t = scr.tile([P, W * 4], F32, tag=tag, name=f"{tag}_{k0}")
                    return t[:].rearrange("p (k b r) -> p k b r", b=2, r=2)

                def t2(tag, dt=F32):
                    t = scr.tile([P, W * 2], dt, tag=tag, name=f"{tag}_{k0}")
                    return t[:].rearrange("p (k c) -> p k c", c=2)

                def t1(tag):
                    return scr.tile([P, W], F32, tag=tag, name=f"{tag}_{k0}")[:]

                hwh = t4("hwh")
                act.mul(hwh, wh, 3.5)
                lt = t4("lt")
                vec.tensor_tensor(out=lt, in0=xy, in1=hwh, op=AL.subtract)
                rb = t4("rb")
                vec.tensor_tensor(out=rb, in0=xy, in1=hwh, op=AL.add)

                LTt = pair(1, k0, W).unsqueeze(2).to_broadcast([P, W, 2, 2])
                RBt = pair(2, k0, W).unsqueeze(2).to_broadcast([P, W, 2, 2])
                wih = t4("wih")
                vec.tensor_tensor(out=wih, in0=rb, in1=RBt, op=AL.min)
                mx = t4("mx")
                vec.tensor_tensor(out=mx, in0=lt, in1=LTt, op=AL.max)
                vec.tensor_tensor(out=wih, in0=wih, in1=mx, op=AL.subtract)
                vec.tensor_scalar_max(out=wih, in0=wih, scalar1=0.0)

                ain = t2("ain")
                vec.tensor_tensor(out=ain, in0=wih[:, :, :, 0], in1=wih[:, :, :, 1],
                                  op=AL.mult)
                atot = t2("atot")
                vec.tensor_tensor(out=atot, in0=wh[:, :, :, 0], in1=wh[:, :, :, 1],
                                  op=AL.mult)
                act.mul(atot, atot, 49.0)
                areab = single(0, k0, W).unsqueeze(2).to_broadcast([P, W, 2])
                vec.tensor_tensor(out=atot, in0=atot, in1=areab, op=AL.add)
                vec.tensor_tensor(out=atot, in0=atot, in1=ain, op=AL.subtract)

                pred = t2("pred")
                vec.tensor_scalar(out=pred, in0=atot, scalar1=1e-6, scalar2=None,
                                  op0=AL.is_gt)
                vec.tensor_scalar_max(out=atot, in0=atot, scalar1=1e-6)
                vec.reciprocal(out=atot, in_=atot)
                iou = t2("iou")
                vec.tensor_tensor(out=iou, in0=ain, in1=atot, op=AL.mult)
                vec.tensor_tensor(out=iou, in0=iou, in1=pred, op=AL.mult)

                # selection (f32 0/1), broadcast over the inner pair dim
                sel2 = t2("sel2")
                i1 = iou[:, :, 1:2].to_broadcast([P, W, 2])
                i0 = iou[:, :, 0:1].to_broadcast([P, W, 2])
                vec.tensor_tensor(out=sel2, in0=i1, in1=i0, op=AL.is_gt)

                def pick2(v3, tag):
                    # v_r = v0 + sel*(v1-v0) over [P,W,2]
                    t = t2(tag)
                    vec.tensor_tensor(out=t, in0=v3[:, :, 1, :], in1=v3[:, :, 0, :],
                                      op=AL.subtract)
                    vec.tensor_tensor(out=t, in0=t, in1=sel2, op=AL.mult)
                    vec.tensor_tensor(out=t, in0=t, in1=v3[:, :, 0, :], op=AL.add)
                    return t

                xyr = pick2(xy, "xyr")
                whr = pick2(wh, "whr")
                cr = t1("cr")
                vec.tensor_tensor(out=cr, in0=cb[:, :, 1], in1=cb[:, :, 0], op=AL.subtract)
                vec.tensor_tensor(out=cr, in0=cr, in1=sel2[:, :, 0], op=AL.mult)
                vec.tensor_tensor(out=cr, in0=cr, in1=cb[:, :, 0], op=AL.add)

                # S4 = (xy_t - xyr)^2 and (ssq_t - ssq(whr))^2, summed 4-wide
                dxy = t2("dxy")
                vec.tensor_tensor(out=dxy, in0=pair(0, k0, W), in1=xyr, op=AL.subtract)
                vec.tensor_tensor(out=dxy, in0=dxy, in1=dxy, op=AL.mult)

                sq_ = t2("sq_")
                sg_ = t2("sg_")
                act.activation(sq_, whr, ACT.Abs)
                act.activation(sq_, sq_, ACT.Sqrt, bias=eps_t[:])
                act.activation(sg_, whr, ACT.Sign)
                vec.tensor_tensor(out=sq_, in0=sq_, in1=sg_, op=AL.mult)
                vec.tensor_tensor(out=sq_, in0=pair(3, k0, W), in1=sq_, op=AL.subtract)
                vec.tensor_tensor(out=sq_, in0=sq_, in1=sq_, op=AL.mult)

                vec.tensor_tensor(out=dxy, in0=dxy, in1=sq_, op=AL.add)
                L = t1("L")
                vec.tensor_reduce(out=L, in_=dxy, axis=X, op=AL.add)
                act.mul(L, L, 5.0)

                # obj: + (cr-1)^2 - 0.5*cr^2
                o1 = t1("o1")
                act.activation(o1, cr, ACT.Square, bias=neg1_t[:])   # (cr-1)^2
                vec.tensor_tensor(out=L, in0=L, in1=o1, op=AL.add)
                act.activation(o1, cr, ACT.Square)                   # cr^2
                vec.tensor_scalar_mul(out=o1, in0=o1, scalar1=0.5)
                vec.tensor_tensor(out=L, in0=L, in1=o1, op=AL.subtract)

                # class terms: + sum(cls^2) + (1 - 2*cls_r)
                big = scr.tile([P, W * 20], F32, tag="big")
                big3 = big[:].rearrange("p (k c) -> p k c", c=20)
                vec.tensor_tensor(
                    out=big3,
                    in0=iotf.unsqueeze(1).to_broadcast([P, W, 20]),
                    in1=single(1, k0, W).unsqueeze(2).to_broadcast([P, W, 20]),
                    op=AL.is_equal)
                vec.tensor_tensor(out=big3, in0=big3, in1=clsg, op=AL.mult)
                vec.tensor_reduce(out=o1, in_=big3, axis=X, op=AL.add)
                vec.tensor_scalar(out=o1, in0=o1, scalar1=-2.0, scalar2=1.0,
                                  op0=AL.mult, op1=AL.add)
                vec.tensor_tensor(out=L, in0=L, in1=o1, op=AL.add)
                vec.tensor_tensor(out=big3, in0=clsg, in1=clsg, op=AL.mult)
                vec.tensor_reduce(out=o1, in_=big3, axis=X, op=AL.add)
                vec.tensor_tensor(out=L, in0=L, in1=o1, op=AL.add)

                # accumulate into acc2[:,0]
                vec.tensor_reduce(out=o1[:, :1], in_=L, axis=X, op=AL.add)
                vec.tensor_tensor(out=acc2[:, 0:1], in0=acc2[:, 0:1], in1=o1[:, :1],
                                  op=AL.add)
                k0 += W

            nc.sync.dma_start(out=out[:], in_=acc2[:])
    nc.compile()
    return nc


def _get_nc():
    if "nc" not in _cache:
        _cache["nc"] = _build()
    return _cache["nc"]


def _host_prep(output, target):
    f32 = np.float32
    out_flat = np.ascontiguousarray(output.reshape(CELLS, D))

    bid = target[:, 7].astype(np.int64)
    gx = target[:, 4].astype(np.int64)
    gy = target[:, 5].astype(np.int64)
    cell = (bid * (GRID * GRID) + gx * GRID + gy).astype(np.int32)

    order = np.argsort(cell, kind="stable")
    ts = target[order]
    cs = cell[order]

    x = ts[:, 0].astype(f32)
    y = ts[:, 1].astype(f32)
    w_ = ts[:, 2].astype(f32)
    h_ = ts[:, 3].astype(f32)
    c35 = f32(3.5)
    ssqw = (np.sign(w_) * np.sqrt(np.abs(w_) + f32(1e-6))).astype(f32)
    ssqh = (np.sign(h_) * np.sqrt(np.abs(h_) + f32(1e-6))).astype(f32)
    lef = (x - c35 * w_).astype(f32)
    rig = (x + c35 * w_).astype(f32)
    top = (y - c35 * h_).astype(f32)
    bot = (y + c35 * h_).astype(f32)
    area = ((w_ * h_) * f32(49.0)).astype(f32)
    cls = ts[:, 6].astype(f32)

    # pairs: XY, LT, RB, SSQ ; singles: AREA, CLS
    pairs = np.stack([
        np.stack([x, y], -1),
        np.stack([lef, top], -1),
        np.stack([rig, bot], -1),
        np.stack([ssqw, ssqh], -1),
    ])                                              # [4, NTGT, 2]
    singles = np.stack([area, cls])                 # [2, NTGT]

    in_maps = []
    for c in range(8):
        lo, hi = c * TGT_CORE, (c + 1) * TGT_CORE
        idx_np = np.ascontiguousarray(cs[lo:hi].reshape(NCH, P).T)        # [P, NCH]
        pr = pairs[:, lo:hi].reshape(4, NCH, P, 2).transpose(2, 0, 1, 3)  # [P,4,NCH,2]
        sg = singles[:, lo:hi].reshape(2, NCH, P).transpose(2, 0, 1)      # [P,2,NCH]
        f_np = np.empty((P, NF * NCH + 20), dtype=f32)
        f_np[:, :8 * NCH] = pr.reshape(P, 8 * NCH)
        f_np[:, 8 * NCH:9 * NCH] = sg[:, 0]
        f_np[:, 9 * NCH:10 * NCH] = sg[:, 1]
        f_np[:, 10 * NCH:] = np.arange(20, dtype=f32)[None, :]
        sl_np = out_flat[c * CELLS_CORE:(c + 1) * CELLS_CORE]
        in_maps.append({
            "table": out_flat,
            "sl": np.ascontiguousarray(sl_np),
            "idx": idx_np.astype(np.int32),
            "fld": np.ascontiguousarray(f_np),
        })
    return in_maps


def _reduce(results):
    tot = 0.0
    for res in results:
        p = res["partial"]
        tot += float(p[:, 0].sum()) + 0.5 * float(p[:, 1].sum())
    return np.float32(tot)


def run(output, target, trace=False, trace_cores=None):
    from concourse.bass_utils import run_bass_kernel_spmd

    nc = _get_nc()
    in_maps = _host_prep(np.asarray(output), np.asarray(target))
    r = run_bass_kernel_spmd(nc, in_maps, core_ids=list(range(8)), trace=trace,
                             trace_cores=trace_cores)
    return _reduce(r.results), r


def kernel(output, target):
    return run(output, target)[0]

